# revision 37
# baseline (speedup 1.0000x reference)
"""AnomalyAttention Trainium2 kernel — 8 NeuronCores, data-parallel over batch.

Computes, for B=32, L=512, H=8, E=64 (shapes hardcoded):
    scores   = Q @ K^T (causal masked), series = softmax(scores/8)
    V_out    = series @ V
    prior    = 1/(sqrt(2pi) sig) * exp(-(i-j)^2 / (2 sig^2))
    sigma_out= broadcast(sig)  with sig = 3^(sigmoid(5*sigma)+1e-5) - 1

Each of the 8 cores handles 4 batches; all heads computed locally, no
collectives. The host pre-transposes Q/K to [E, L] layout and casts Q/K/V to
bf16 (TensorEngine compute dtype), and appends a ones-column per head to V so
the series@V matmul also produces the softmax row-sums. On-chip: QK^T and
series@V on the TensorEngine (fp32 accumulation), exp/prior on the
ScalarEngine, normalization/masking/sigma-broadcast on the VectorEngine.
Input slabs load through the gpsimd software-DGE queue so they never sit
behind the output stream on the two hardware DMA queues (SP and ACT).
sigma_out is written with 4 rows packed per SBUF partition, which makes the
DMA destination runs 8 KiB instead of 2 KiB. The causal structure skips all
fully-masked score blocks; their series output stays zero because the runner
donates pre-zeroed output buffers.
"""
import math
import sys
import types
from contextlib import ExitStack

sys.path.insert(0, "/opt/trn_rl_repo")

import numpy as np

# NTFF profile hook shim: the container's antenv package lacks axon_hooks, so
# register an equivalent module before concourse imports it (trace=True path).
if "antenv.axon_hooks" not in sys.modules:
    _hook_mod = types.ModuleType("antenv.axon_hooks")
    _hook_store = [None]
    _hook_mod.set_axon_ntff_profile_hook = lambda h: _hook_store.__setitem__(0, h)
    _hook_mod.get_axon_ntff_profile_hook = lambda: _hook_store[0]
    sys.modules["antenv.axon_hooks"] = _hook_mod
    try:
        import antenv

        antenv.axon_hooks = _hook_mod
        from trn_agent_boot.trn_boot import _ntff_profile_via_ctypes

        _hook = _ntff_profile_via_ctypes("/opt/axon/libaxon_pjrt.so")
        if _hook is not None:
            _hook_mod.set_axon_ntff_profile_hook(_hook)
    except Exception:
        pass

import ml_dtypes
import concourse.bass as bass
import concourse.tile as tile
from concourse import mybir
from concourse.bass_utils import run_bass_kernel_spmd
from concourse.masks import make_identity

F32 = mybir.dt.float32
BF16 = mybir.dt.bfloat16
ACT = mybir.ActivationFunctionType
MUL = mybir.AluOpType.mult

N_CORES = 8
B, L, H, E = 32, 512, 8, 64
BL = B // N_CORES  # batches per core
P = 128
NB = L // P  # 4 row blocks
RPP = L // P  # rows packed per partition for sigma_out (4)
HP = H // 2  # head pairs (two heads' E dims stacked on 128 partitions)
E1 = E + 1  # V columns per head incl. the ones column
LN3 = math.log(3.0)
NEG_HALF_LN_2PI = -0.5 * math.log(2.0 * math.pi)
# prior band per row block: 64-aligned columns covering |i-j| <= 33
PB0 = [0, 64, 192, 320]
PBW = [192, 256, 256, 192]


def _split_excess_waits(nc):
    """This container's walrus accepts at most one sync-wait per instruction
    (two for EventSemaphore), but Tile attaches one wait per dependency.
    Hoist excess waits onto dedicated same-engine NOPs placed immediately
    before the instruction — equivalent for monotone (sem-ge) waits."""
    fixn = [0]
    for f in nc.m.functions:
        for bb in f.blocks:
            out = []
            changed = False
            for inst in bb.instructions:
                si = inst.sync_info
                n = len(si.on_wait) if si and si.on_wait else 0
                cap = 2 if isinstance(inst, mybir.InstEventSemaphore) else 1
                if n > cap:
                    waits = list(si.on_wait)
                    # keep non-monotone (eq) waits on the instruction itself
                    waits.sort(key=lambda w: "ge" in w.wait_mode)
                    keep, hoist = waits[:cap], waits[cap:]
                    for w in hoist:
                        assert "ge" in w.wait_mode, w
                        fixn[0] += 1
                        nop = mybir.InstNoOp(
                            name=f"Iwfix-{fixn[0]}",
                            engine=inst.engine,
                            ins=[],
                            outs=[],
                            bass_nofuse=True,
                        )
                        nop.sync_info = type(si)(on_wait=[w], on_update=[])
                        out.append(nop)
                    si.on_wait = keep
                    changed = True
                out.append(inst)
            if changed:
                bb.instructions = out
    return nc


def _bcast(ap, n):
    """Append a stride-0 dimension of size n to an AP (free-dim broadcast)."""
    return bass.AP(tensor=ap.tensor, offset=ap.offset, ap=[*ap.ap, [0, n]])


def _build():
    nc = bass.Bass("TRN2")
    qt = nc.declare_dram_parameter("qt", [BL, HP, P, L], BF16, isOutput=False)
    kt = nc.declare_dram_parameter("kt", [BL, HP, P, L], BF16, isOutput=False)
    v5 = nc.declare_dram_parameter("v5", [BL, L, H * E1], BF16, isOutput=False)
    # host-computed sigma-derived row scalars (0.06% of the elements):
    # sig, -1/(2 sig^2), ln(1/(sqrt(2pi) sig)), and sig packed 4-rows/partition
    sgs = nc.declare_dram_parameter("sgs", [BL, L, H], F32, isOutput=False)
    sgn = nc.declare_dram_parameter("sgn", [BL, L, H], F32, isOutput=False)
    sgl = nc.declare_dram_parameter("sgl", [BL, L, H], F32, isOutput=False)
    sg4 = nc.declare_dram_parameter("sg4", [BL, P, RPP, H], F32, isOutput=False)
    d2 = nc.declare_dram_parameter("d2", [L, L], F32, isOutput=False)
    vo = nc.declare_dram_parameter("vo", [BL, L, H * E], F32, isOutput=True)
    so = nc.declare_dram_parameter("so", [BL, H, L, L], F32, isOutput=True)
    po = nc.declare_dram_parameter("po", [BL, H, L, L], F32, isOutput=True)
    go = nc.declare_dram_parameter("go", [BL, H, L, L], F32, isOutput=True)

    with ExitStack() as ctx:
        tc = ctx.enter_context(tile.TileContext(nc))
        consts = ctx.enter_context(tc.tile_pool(name="consts", bufs=1))
        sparams = ctx.enter_context(tc.tile_pool(name="sparams", bufs=1))
        slabs = ctx.enter_context(tc.tile_pool(name="slabs", bufs=2))
        work = ctx.enter_context(tc.tile_pool(name="work", bufs=5))
        gop = ctx.enter_context(tc.tile_pool(name="gop", bufs=3))
        bigp = ctx.enter_context(tc.tile_pool(name="bigp", bufs=2))
        eTp = ctx.enter_context(tc.tile_pool(name="eTp", bufs=10))
        small = ctx.enter_context(tc.tile_pool(name="small", bufs=12))
        ps_sc = ctx.enter_context(tc.tile_pool(name="ps_sc", bufs=4, space="PSUM"))
        ps_tr = ctx.enter_context(tc.tile_pool(name="ps_tr", bufs=2, space="PSUM"))
        ps_av = ctx.enter_context(tc.tile_pool(name="ps_av", bufs=2, space="PSUM"))

        ident = consts.tile([P, P], BF16)
        make_identity(nc, ident)
        # multiplicative causal masks: trimask[q,s]=1 iff s<=q (diag block of
        # the [q,s] layout); trimaskT[s,q]=1 iff s<=q (its transpose)
        trimask = consts.tile([P, P], BF16)
        nc.gpsimd.memset(trimask, 1.0)
        nc.gpsimd.affine_select(
            out=trimask,
            in_=trimask,
            compare_op=mybir.AluOpType.is_ge,
            fill=0.0,
            base=0,
            pattern=[[-1, P]],
            channel_multiplier=1,
        )
        trimaskT = consts.tile([P, P], BF16)
        nc.gpsimd.memset(trimaskT, 1.0)
        # keep where s<=q in [s(part), q(free)] layout: (q - s) >= 0
        nc.gpsimd.affine_select(
            out=trimaskT,
            in_=trimaskT,
            compare_op=mybir.AluOpType.is_ge,
            fill=0.0,
            base=0,
            pattern=[[1, P]],
            channel_multiplier=-1,
        )
        # full-width causal keep-masks per row block: crow[i][q, s] = 1 iff
        # s <= 128*i + q  (serf's mask, applied in one fused op per tile)
        crow = []
        for i in range(NB):
            cm = consts.tile([P, (i + 1) * P], BF16, tag=f"crow{i}")
            nc.gpsimd.memset(cm, 1.0)
            nc.gpsimd.affine_select(
                out=cm,
                in_=cm,
                compare_op=mybir.AluOpType.is_ge,
                fill=0.0,
                base=i * P,
                pattern=[[-1, (i + 1) * P]],
                channel_multiplier=1,
            )
            crow.append(cm)
        def load_slabs(b, eng, veng=None):
            # pre-transposed Q/K and ones-augmented V (bf16)
            veng = veng or eng
            qT, kT = [], []
            for hp in range(HP):
                qTt = slabs.tile([P, L], BF16, tag=f"qT{hp}")
                eng.dma_start(out=qTt, in_=qt[b, hp])
                qT.append(qTt)
                kTt = slabs.tile([P, L], BF16, tag=f"kT{hp}")
                eng.dma_start(out=kTt, in_=kt[b, hp])
                kT.append(kTt)
            vb = []
            for t in range(NB):
                vt = slabs.tile([P, H * E1], BF16, tag=f"v{t}")
                veng.dma_start(out=vt, in_=v5[b, t * P : (t + 1) * P, :])
                vb.append(vt)
            return qT, kT, vb

        def load_sparams(b, eng):
            s4 = sparams.tile([P, RPP, H], F32, tag=f"sig4_{b}")
            eng.dma_start(out=s4, in_=sg4[b])
            out = [s4]
            for src, nm in ((sgs, "sig"), (sgn, "ns"), (sgl, "lc")):
                tl = sparams.tile([P, NB, H], F32, tag=f"{nm}{b}")
                eng.dma_start(
                    out=tl, in_=src[b].rearrange("(t p) h -> p t h", p=P)
                )
                out.append(tl)
            return out

        # batch 0's inputs split across the (empty at t=0) SP hardware queue
        # and the gpsimd software queue; later batches prefetch through the
        # software queue mid-way through the previous batch so they never sit
        # behind the output stream.
        slabs_next = load_slabs(0, nc.sync, veng=nc.gpsimd)
        d2t = []
        for i in range(NB):
            t = consts.tile([P, L], F32, tag=f"d2_{i}")
            nc.sync.dma_start(out=t, in_=d2[i * P : (i + 1) * P, :])
            d2t.append(t)
        sp0 = load_sparams(0, nc.sync)
        sig4 = [None] * BL
        sig_a = [None] * BL
        ns_a = [None] * BL
        lc_a = [None] * BL
        sig4[0], sig_a[0], ns_a[0], lc_a[0] = sp0
        for b in range(1, BL):
            sig4[b], sig_a[b], ns_a[b], lc_a[b] = load_sparams(b, nc.gpsimd)

        for b in range(BL):
            qT, kT, vb = slabs_next

            # ---- per (row-block, head) attention + prior ----
            # series rows beyond the causal width are never written — the
            # runner donates pre-zeroed output buffers, so they stay 0.
            for gi, i in enumerate(reversed(range(NB))):
                W = (i + 1) * P  # causal width of this row block
                rows = slice(i * P, (i + 1) * P)
                vos_all = work.tile([P, H * E], F32, tag="voall")
                serf_all = bigp.tile([P, H, L], F32, tag="serall")
                pri_all = bigp.tile([P, H, 256], F32, tag="priall")
                for h in range(H):
                    hp, ho = h // 2, (h % 2) * E
                    sc = ps_sc.tile([P, L], F32, tag="sc")
                    nc.tensor.matmul(
                        sc[:, :W],
                        lhsT=qT[hp][ho : ho + E, rows],
                        rhs=kT[hp][ho : ho + E, :W],
                        start=True,
                        stop=True,
                    )
                    expb = work.tile([P, L], BF16, tag="exp")
                    nc.scalar.activation(
                        out=expb[:, :W], in_=sc[:, :W], func=ACT.Exp, scale=0.125
                    )
                    # series @ [V | 1]: transposed exp blocks; col E is the
                    # row-sum. The diagonal chunk's upper triangle is zeroed
                    # during the PSUM->SBUF copy (mask folded into the mul).
                    eTs = []
                    for j in range(i + 1):
                        pt = ps_tr.tile([P, P], BF16, tag="ps_tr")
                        nc.tensor.transpose(pt, expb[:, j * P : (j + 1) * P], ident)
                        eT = eTp.tile([P, P], BF16, tag="eT")
                        if j == i:
                            nc.vector.tensor_mul(out=eT, in0=pt, in1=trimaskT)
                        else:
                            nc.vector.tensor_copy(out=eT, in_=pt)
                        eTs.append(eT)
                    va = ps_av.tile([P, E1], F32, tag="va")
                    for j in range(i + 1):
                        nc.tensor.matmul(
                            va,
                            lhsT=eTs[j],
                            rhs=vb[j][:, h * E1 : (h + 1) * E1],
                            start=(j == 0),
                            stop=(j == i),
                        )
                    rinv = small.tile([P, 1], F32, tag="rinv")
                    nc.vector.reciprocal(out=rinv, in_=va[:, E : E + 1])
                    nc.vector.scalar_tensor_tensor(
                        out=serf_all[:, h, :W],
                        in0=expb[:, :W],
                        scalar=rinv,
                        in1=crow[i],
                        op0=MUL,
                        op1=MUL,
                    )
                    nc.vector.tensor_scalar_mul(
                        out=vos_all[:, h * E : (h + 1) * E],
                        in0=va[:, :E],
                        scalar1=rinv,
                    )
                    # prior: exp(d2 * (-1/(2 sig^2)) + ln c) in one ACT pass.
                    # sig <= 2.0002 hard-bounds the Gaussian: fp32-exact 0
                    # beyond |i-j| > 33, so only an aligned band is written
                    # (the donated output buffer is pre-zeroed elsewhere).
                    c0, wp = PB0[i], PBW[i]
                    nc.scalar.activation(
                        out=pri_all[:, h, :wp],
                        in_=d2t[i][:, c0 : c0 + wp],
                        func=ACT.Exp,
                        scale=ns_a[b][:, i, h : h + 1],
                        bias=lc_a[b][:, i, h : h + 1],
                    )
                    # sigma_out (4 rows per partition -> 8 KiB DMA runs);
                    # two heads' worth per row-block to spread the load —
                    # one broadcast on the vector engine, one on gpsimd
                    if h == 2 * i:
                        for hh in (2 * i, 2 * i + 1):
                            sgo = gop.tile([P, RPP, L], F32, tag="sgo")
                            src = _bcast(sig4[b][:, :, hh], L)
                            if hh % 2 == 0:
                                nc.vector.tensor_copy(out=sgo, in_=src)
                            else:
                                nc.scalar.activation(
                                    out=sgo, in_=src, func=ACT.Copy
                                )
                            eng = nc.sync
                            eng.dma_start(
                                out=go[b, hh].rearrange(
                                    "(p k) s -> p (k s)", k=RPP
                                ),
                                in_=sgo,
                            )
                nc.sync.dma_start(
                    out=so[b].rearrange("h r w -> r h w")[rows, :, :W],
                    in_=serf_all[:, :, :W],
                )
                c0, wp = PB0[i], PBW[i]
                nc.sync.dma_start(
                    out=po[b].rearrange("h r w -> r h w")[rows, :, c0 : c0 + wp],
                    in_=pri_all[:, :, :wp],
                )
                nc.sync.dma_start(out=vo[b, rows, :], in_=vos_all)
                if gi == 0 and b + 1 < BL:
                    # prefetch the next batch's slabs ahead of this batch's
                    # remaining sigma_out transfers in the software queue
                    slabs_next = load_slabs(b + 1, nc.gpsimd)
    return _split_excess_waits(nc)


_nc_cache = None
last_results = None


def kernel(queries, keys, values, sigma, attention_mask=None, **_unused):
    """Full-input entry point: shard over 8 cores, run, gather."""
    global _nc_cache, last_results
    if _nc_cache is None:
        _nc_cache = _build()
    nc = _nc_cache

    queries = np.ascontiguousarray(np.asarray(queries), dtype=np.float32)
    keys = np.ascontiguousarray(np.asarray(keys), dtype=np.float32)
    values = np.ascontiguousarray(np.asarray(values), dtype=np.float32)
    sigma = np.ascontiguousarray(np.asarray(sigma), dtype=np.float32)

    bf = ml_dtypes.bfloat16
    # Q/K transposed to [B, head-pair, 2E, L] so two heads' E dims stack on
    # the 128 SBUF partitions; V gets a ones column per head (row-sum trick).
    qT = np.ascontiguousarray(
        queries.reshape(B, L, HP, 2 * E).transpose(0, 2, 3, 1)
    ).astype(bf)
    kT = np.ascontiguousarray(
        keys.reshape(B, L, HP, 2 * E).transpose(0, 2, 3, 1)
    ).astype(bf)
    v5 = np.ones((B, L, H, E1), dtype=bf)
    v5[..., :E] = values.reshape(B, L, H, E).astype(bf)
    v5 = v5.reshape(B, L, H * E1)
    # sigma-derived row scalars (tiny: B*L*H elements)
    s1 = (1.0 / (1.0 + np.exp(-5.0 * sigma)) + 1e-5).astype(np.float32)
    sig = (np.power(3.0, s1, dtype=np.float32) - 1.0).astype(np.float32)
    sgn = (-0.5 / (sig * sig)).astype(np.float32)
    sgl = (-np.log(sig) + np.float32(NEG_HALF_LN_2PI)).astype(np.float32)
    sg4 = sig.reshape(B, P, RPP, H)

    idx = np.arange(L, dtype=np.float32)
    d2 = (idx[:, None] - idx[None, :]) ** 2

    in_maps = []
    for c in range(N_CORES):
        bs = slice(c * BL, (c + 1) * BL)
        in_maps.append(
            {
                "qt": qT[bs],
                "kt": kT[bs],
                "v5": v5[bs],
                "sgs": sig[bs],
                "sgn": sgn[bs],
                "sgl": sgl[bs],
                "sg4": sg4[bs],
                "d2": d2,
            }
        )

    res = run_bass_kernel_spmd(nc, in_maps, core_ids=list(range(N_CORES)))
    last_results = res

    V = np.concatenate(
        [res.results[c]["vo"].reshape(BL, L, H, E) for c in range(N_CORES)], axis=0
    )
    series = np.concatenate([res.results[c]["so"] for c in range(N_CORES)], axis=0)
    prior = np.concatenate([res.results[c]["po"] for c in range(N_CORES)], axis=0)
    sigma_out = np.concatenate([res.results[c]["go"] for c in range(N_CORES)], axis=0)
    return V, series, prior, sigma_out


# revision 38
# speedup vs baseline: 1.1903x; 1.1903x over previous
"""AnomalyAttention Trainium2 kernel — 8 NeuronCores, data-parallel over batch.

Computes, for B=32, L=512, H=8, E=64 (shapes hardcoded):
    scores   = Q @ K^T (causal masked), series = softmax(scores/8)
    V_out    = series @ V
    prior    = 1/(sqrt(2pi) sig) * exp(-(i-j)^2 / (2 sig^2))
    sigma_out= broadcast(sig)  with sig = 3^(sigmoid(5*sigma)+1e-5) - 1

Each of the 8 cores handles 4 batches; all heads computed locally, no
collectives. The host pre-transposes Q/K to [E, L] layout and casts Q/K/V to
bf16 (TensorEngine compute dtype), and appends a ones-column per head to V so
the series@V matmul also produces the softmax row-sums. On-chip: QK^T and
series@V on the TensorEngine (fp32 accumulation), exp/prior on the
ScalarEngine, normalization/masking/sigma-broadcast on the VectorEngine.
Input slabs load through the gpsimd software-DGE queue so they never sit
behind the output stream on the two hardware DMA queues (SP and ACT).
sigma_out is written with 4 rows packed per SBUF partition, which makes the
DMA destination runs 8 KiB instead of 2 KiB. The causal structure skips all
fully-masked score blocks; their series output stays zero because the runner
donates pre-zeroed output buffers.
"""
import math
import sys
import types
from contextlib import ExitStack

sys.path.insert(0, "/opt/trn_rl_repo")

import numpy as np

# NTFF profile hook shim: the container's antenv package lacks axon_hooks, so
# register an equivalent module before concourse imports it (trace=True path).
if "antenv.axon_hooks" not in sys.modules:
    _hook_mod = types.ModuleType("antenv.axon_hooks")
    _hook_store = [None]
    _hook_mod.set_axon_ntff_profile_hook = lambda h: _hook_store.__setitem__(0, h)
    _hook_mod.get_axon_ntff_profile_hook = lambda: _hook_store[0]
    sys.modules["antenv.axon_hooks"] = _hook_mod
    try:
        import antenv

        antenv.axon_hooks = _hook_mod
        from trn_agent_boot.trn_boot import _ntff_profile_via_ctypes

        _hook = _ntff_profile_via_ctypes("/opt/axon/libaxon_pjrt.so")
        if _hook is not None:
            _hook_mod.set_axon_ntff_profile_hook(_hook)
    except Exception:
        pass

import ml_dtypes
import concourse.bass as bass
import concourse.tile as tile
from concourse import mybir
from concourse.bass_utils import run_bass_kernel_spmd
from concourse.masks import make_identity

F32 = mybir.dt.float32
BF16 = mybir.dt.bfloat16
ACT = mybir.ActivationFunctionType
MUL = mybir.AluOpType.mult

N_CORES = 8
B, L, H, E = 32, 512, 8, 64
BL = B // N_CORES  # batches per core
P = 128
NB = L // P  # 4 row blocks
RPP = L // P  # rows packed per partition for sigma_out (4)
HP = H // 2  # head pairs (two heads' E dims stacked on 128 partitions)
E1 = E + 1  # V columns per head incl. the ones column
LN3 = math.log(3.0)
NEG_HALF_LN_2PI = -0.5 * math.log(2.0 * math.pi)
# prior band per row block: 64-aligned columns covering |i-j| <= 33
PB0 = [0, 64, 192, 320]
PBW = [192, 256, 256, 192]


def _split_excess_waits(nc):
    """This container's walrus accepts at most one sync-wait per instruction
    (two for EventSemaphore), but Tile attaches one wait per dependency.
    Hoist excess waits onto dedicated same-engine NOPs placed immediately
    before the instruction — equivalent for monotone (sem-ge) waits."""
    fixn = [0]
    for f in nc.m.functions:
        for bb in f.blocks:
            out = []
            changed = False
            for inst in bb.instructions:
                si = inst.sync_info
                n = len(si.on_wait) if si and si.on_wait else 0
                cap = 2 if isinstance(inst, mybir.InstEventSemaphore) else 1
                if n > cap:
                    waits = list(si.on_wait)
                    # keep non-monotone (eq) waits on the instruction itself
                    waits.sort(key=lambda w: "ge" in w.wait_mode)
                    keep, hoist = waits[:cap], waits[cap:]
                    for w in hoist:
                        assert "ge" in w.wait_mode, w
                        fixn[0] += 1
                        nop = mybir.InstNoOp(
                            name=f"Iwfix-{fixn[0]}",
                            engine=inst.engine,
                            ins=[],
                            outs=[],
                            bass_nofuse=True,
                        )
                        nop.sync_info = type(si)(on_wait=[w], on_update=[])
                        out.append(nop)
                    si.on_wait = keep
                    changed = True
                out.append(inst)
            if changed:
                bb.instructions = out
    return nc


def _bcast(ap, n):
    """Append a stride-0 dimension of size n to an AP (free-dim broadcast)."""
    return bass.AP(tensor=ap.tensor, offset=ap.offset, ap=[*ap.ap, [0, n]])


def _build():
    nc = bass.Bass("TRN2")
    qt = nc.declare_dram_parameter("qt", [BL, HP, P, L], BF16, isOutput=False)
    kt = nc.declare_dram_parameter("kt", [BL, HP, P, L], BF16, isOutput=False)
    v5 = nc.declare_dram_parameter("v5", [BL, L, H * E1], BF16, isOutput=False)
    # host-computed sigma-derived row scalars (0.06% of the elements):
    # sig, -1/(2 sig^2), ln(1/(sqrt(2pi) sig)), and sig packed 4-rows/partition
    sgs = nc.declare_dram_parameter("sgs", [BL, L, H], F32, isOutput=False)
    sgn = nc.declare_dram_parameter("sgn", [BL, L, H], F32, isOutput=False)
    sgl = nc.declare_dram_parameter("sgl", [BL, L, H], F32, isOutput=False)
    sg4 = nc.declare_dram_parameter("sg4", [BL, P, RPP, H], F32, isOutput=False)
    d2 = nc.declare_dram_parameter("d2", [L, L], F32, isOutput=False)
    vo = nc.declare_dram_parameter("vo", [BL, L, H * E], F32, isOutput=True)
    so = nc.declare_dram_parameter("so", [BL, H, L, L], F32, isOutput=True)
    po = nc.declare_dram_parameter("po", [BL, H, L, L], F32, isOutput=True)
    go = nc.declare_dram_parameter("go", [BL, H, L, L], F32, isOutput=True)

    with ExitStack() as ctx:
        tc = ctx.enter_context(tile.TileContext(nc))
        consts = ctx.enter_context(tc.tile_pool(name="consts", bufs=1))
        sparams = ctx.enter_context(tc.tile_pool(name="sparams", bufs=1))
        slabs = ctx.enter_context(tc.tile_pool(name="slabs", bufs=2))
        work = ctx.enter_context(tc.tile_pool(name="work", bufs=5))
        gop = ctx.enter_context(tc.tile_pool(name="gop", bufs=3))
        bigp = ctx.enter_context(tc.tile_pool(name="bigp", bufs=2))
        eTp = ctx.enter_context(tc.tile_pool(name="eTp", bufs=10))
        small = ctx.enter_context(tc.tile_pool(name="small", bufs=12))
        ps_sc = ctx.enter_context(tc.tile_pool(name="ps_sc", bufs=4, space="PSUM"))
        ps_tr = ctx.enter_context(tc.tile_pool(name="ps_tr", bufs=2, space="PSUM"))
        ps_av = ctx.enter_context(tc.tile_pool(name="ps_av", bufs=2, space="PSUM"))

        ident = consts.tile([P, P], BF16)
        make_identity(nc, ident)
        # multiplicative causal masks: trimask[q,s]=1 iff s<=q (diag block of
        # the [q,s] layout); trimaskT[s,q]=1 iff s<=q (its transpose)
        trimask = consts.tile([P, P], BF16)
        nc.gpsimd.memset(trimask, 1.0)
        nc.gpsimd.affine_select(
            out=trimask,
            in_=trimask,
            compare_op=mybir.AluOpType.is_ge,
            fill=0.0,
            base=0,
            pattern=[[-1, P]],
            channel_multiplier=1,
        )
        trimaskT = consts.tile([P, P], BF16)
        nc.gpsimd.memset(trimaskT, 1.0)
        # keep where s<=q in [s(part), q(free)] layout: (q - s) >= 0
        nc.gpsimd.affine_select(
            out=trimaskT,
            in_=trimaskT,
            compare_op=mybir.AluOpType.is_ge,
            fill=0.0,
            base=0,
            pattern=[[1, P]],
            channel_multiplier=-1,
        )
        # full-width causal keep-masks per row block: crow[i][q, s] = 1 iff
        # s <= 128*i + q  (serf's mask, applied in one fused op per tile)
        crow = []
        for i in range(NB):
            cm = consts.tile([P, (i + 1) * P], BF16, tag=f"crow{i}")
            nc.gpsimd.memset(cm, 1.0)
            nc.gpsimd.affine_select(
                out=cm,
                in_=cm,
                compare_op=mybir.AluOpType.is_ge,
                fill=0.0,
                base=i * P,
                pattern=[[-1, (i + 1) * P]],
                channel_multiplier=1,
            )
            crow.append(cm)
        def load_slabs(b, eng, veng=None):
            # pre-transposed Q/K and ones-augmented V (bf16)
            veng = veng or eng
            qT, kT = [], []
            for hp in range(HP):
                qTt = slabs.tile([P, L], BF16, tag=f"qT{hp}")
                eng.dma_start(out=qTt, in_=qt[b, hp])
                qT.append(qTt)
                kTt = slabs.tile([P, L], BF16, tag=f"kT{hp}")
                eng.dma_start(out=kTt, in_=kt[b, hp])
                kT.append(kTt)
            vb = []
            for t in range(NB):
                vt = slabs.tile([P, H * E1], BF16, tag=f"v{t}")
                veng.dma_start(out=vt, in_=v5[b, t * P : (t + 1) * P, :])
                vb.append(vt)
            return qT, kT, vb

        def load_sparams(b, eng):
            s4 = sparams.tile([P, RPP, H], F32, tag=f"sig4_{b}")
            eng.dma_start(out=s4, in_=sg4[b])
            out = [s4]
            for src, nm in ((sgs, "sig"), (sgn, "ns"), (sgl, "lc")):
                tl = sparams.tile([P, NB, H], F32, tag=f"{nm}{b}")
                eng.dma_start(
                    out=tl, in_=src[b].rearrange("(t p) h -> p t h", p=P)
                )
                out.append(tl)
            return out

        # batch 0's inputs split across the (empty at t=0) SP hardware queue
        # and the gpsimd software queue; later batches prefetch through the
        # software queue mid-way through the previous batch so they never sit
        # behind the output stream.
        slabs_next = load_slabs(0, nc.sync, veng=nc.gpsimd)
        d2t = []
        for i in range(NB):
            t = consts.tile([P, L], F32, tag=f"d2_{i}")
            nc.sync.dma_start(out=t, in_=d2[i * P : (i + 1) * P, :])
            d2t.append(t)
        sp0 = load_sparams(0, nc.sync)
        sig4 = [None] * BL
        sig_a = [None] * BL
        ns_a = [None] * BL
        lc_a = [None] * BL
        sig4[0], sig_a[0], ns_a[0], lc_a[0] = sp0
        for b in range(1, BL):
            sig4[b], sig_a[b], ns_a[b], lc_a[b] = load_sparams(b, nc.gpsimd)

        for b in range(BL):
            qT, kT, vb = slabs_next

            # ---- per (row-block, head) attention + prior ----
            # series rows beyond the causal width are never written — the
            # runner donates pre-zeroed output buffers, so they stay 0.
            for i in range(NB):
                W = (i + 1) * P  # causal width of this row block
                rows = slice(i * P, (i + 1) * P)
                vos_all = work.tile([P, H * E], F32, tag="voall")
                serf_all = bigp.tile([P, H, L], F32, tag="serall")
                pri_all = bigp.tile([P, H, 256], F32, tag="priall")
                for h in range(H):
                    hp, ho = h // 2, (h % 2) * E
                    sc = ps_sc.tile([P, L], F32, tag="sc")
                    nc.tensor.matmul(
                        sc[:, :W],
                        lhsT=qT[hp][ho : ho + E, rows],
                        rhs=kT[hp][ho : ho + E, :W],
                        start=True,
                        stop=True,
                    )
                    expb = work.tile([P, L], BF16, tag="exp")
                    nc.scalar.activation(
                        out=expb[:, :W], in_=sc[:, :W], func=ACT.Exp, scale=0.125
                    )
                    # series @ [V | 1]: transposed exp blocks; col E is the
                    # row-sum. The diagonal chunk's upper triangle is zeroed
                    # during the PSUM->SBUF copy (mask folded into the mul).
                    eTs = []
                    for j in range(i + 1):
                        pt = ps_tr.tile([P, P], BF16, tag="ps_tr")
                        nc.tensor.transpose(pt, expb[:, j * P : (j + 1) * P], ident)
                        eT = eTp.tile([P, P], BF16, tag="eT")
                        if j == i:
                            nc.vector.tensor_mul(out=eT, in0=pt, in1=trimaskT)
                        else:
                            nc.vector.tensor_copy(out=eT, in_=pt)
                        eTs.append(eT)
                    va = ps_av.tile([P, E1], F32, tag="va")
                    for j in range(i + 1):
                        nc.tensor.matmul(
                            va,
                            lhsT=eTs[j],
                            rhs=vb[j][:, h * E1 : (h + 1) * E1],
                            start=(j == 0),
                            stop=(j == i),
                        )
                    rinv = small.tile([P, 1], F32, tag="rinv")
                    nc.vector.reciprocal(out=rinv, in_=va[:, E : E + 1])
                    nc.vector.scalar_tensor_tensor(
                        out=serf_all[:, h, :W],
                        in0=expb[:, :W],
                        scalar=rinv,
                        in1=crow[i],
                        op0=MUL,
                        op1=MUL,
                    )
                    nc.vector.tensor_scalar_mul(
                        out=vos_all[:, h * E : (h + 1) * E],
                        in0=va[:, :E],
                        scalar1=rinv,
                    )
                    # prior: exp(d2 * (-1/(2 sig^2)) + ln c) in one ACT pass.
                    # sig <= 2.0002 hard-bounds the Gaussian: fp32-exact 0
                    # beyond |i-j| > 33, so only an aligned band is written
                    # (the donated output buffer is pre-zeroed elsewhere).
                    c0, wp = PB0[i], PBW[i]
                    nc.scalar.activation(
                        out=pri_all[:, h, :wp],
                        in_=d2t[i][:, c0 : c0 + wp],
                        func=ACT.Exp,
                        scale=ns_a[b][:, i, h : h + 1],
                        bias=lc_a[b][:, i, h : h + 1],
                    )
                    # sigma_out (4 rows per partition -> 8 KiB DMA runs);
                    # two heads' worth per row-block to spread the load —
                    # one broadcast on the vector engine, one on gpsimd
                    if h == 2 * i:
                        for hh in (2 * i, 2 * i + 1):
                            sgo = gop.tile([P, RPP, L], F32, tag="sgo")
                            src = _bcast(sig4[b][:, :, hh], L)
                            if hh % 2 == 0:
                                nc.vector.tensor_copy(out=sgo, in_=src)
                            else:
                                nc.scalar.activation(
                                    out=sgo, in_=src, func=ACT.Copy
                                )
                            eng = nc.sync
                            eng.dma_start(
                                out=go[b, hh].rearrange(
                                    "(p k) s -> p (k s)", k=RPP
                                ),
                                in_=sgo,
                            )
                nc.sync.dma_start(
                    out=so[b].rearrange("h r w -> r h w")[rows, :, :W],
                    in_=serf_all[:, :, :W],
                )
                c0, wp = PB0[i], PBW[i]
                nc.sync.dma_start(
                    out=po[b].rearrange("h r w -> r h w")[rows, :, c0 : c0 + wp],
                    in_=pri_all[:, :, :wp],
                )
                nc.sync.dma_start(out=vo[b, rows, :], in_=vos_all)
                if i == 0 and b + 1 < BL:
                    # prefetch the next batch's slabs ahead of this batch's
                    # remaining sigma_out transfers in the software queue
                    slabs_next = load_slabs(b + 1, nc.gpsimd)
    return _split_excess_waits(nc)


_nc_cache = None
last_results = None


def kernel(queries, keys, values, sigma, attention_mask=None, **_unused):
    """Full-input entry point: shard over 8 cores, run, gather."""
    global _nc_cache, last_results
    if _nc_cache is None:
        _nc_cache = _build()
    nc = _nc_cache

    queries = np.ascontiguousarray(np.asarray(queries), dtype=np.float32)
    keys = np.ascontiguousarray(np.asarray(keys), dtype=np.float32)
    values = np.ascontiguousarray(np.asarray(values), dtype=np.float32)
    sigma = np.ascontiguousarray(np.asarray(sigma), dtype=np.float32)

    bf = ml_dtypes.bfloat16
    # Q/K transposed to [B, head-pair, 2E, L] so two heads' E dims stack on
    # the 128 SBUF partitions; V gets a ones column per head (row-sum trick).
    qT = np.ascontiguousarray(
        queries.reshape(B, L, HP, 2 * E).transpose(0, 2, 3, 1)
    ).astype(bf)
    kT = np.ascontiguousarray(
        keys.reshape(B, L, HP, 2 * E).transpose(0, 2, 3, 1)
    ).astype(bf)
    v5 = np.ones((B, L, H, E1), dtype=bf)
    v5[..., :E] = values.reshape(B, L, H, E).astype(bf)
    v5 = v5.reshape(B, L, H * E1)
    # sigma-derived row scalars (tiny: B*L*H elements)
    s1 = (1.0 / (1.0 + np.exp(-5.0 * sigma)) + 1e-5).astype(np.float32)
    sig = (np.power(3.0, s1, dtype=np.float32) - 1.0).astype(np.float32)
    sgn = (-0.5 / (sig * sig)).astype(np.float32)
    sgl = (-np.log(sig) + np.float32(NEG_HALF_LN_2PI)).astype(np.float32)
    sg4 = sig.reshape(B, P, RPP, H)

    idx = np.arange(L, dtype=np.float32)
    d2 = (idx[:, None] - idx[None, :]) ** 2

    in_maps = []
    for c in range(N_CORES):
        bs = slice(c * BL, (c + 1) * BL)
        in_maps.append(
            {
                "qt": qT[bs],
                "kt": kT[bs],
                "v5": v5[bs],
                "sgs": sig[bs],
                "sgn": sgn[bs],
                "sgl": sgl[bs],
                "sg4": sg4[bs],
                "d2": d2,
            }
        )

    res = run_bass_kernel_spmd(nc, in_maps, core_ids=list(range(N_CORES)))
    last_results = res

    V = np.concatenate(
        [res.results[c]["vo"].reshape(BL, L, H, E) for c in range(N_CORES)], axis=0
    )
    series = np.concatenate([res.results[c]["so"] for c in range(N_CORES)], axis=0)
    prior = np.concatenate([res.results[c]["po"] for c in range(N_CORES)], axis=0)
    sigma_out = np.concatenate([res.results[c]["go"] for c in range(N_CORES)], axis=0)
    return V, series, prior, sigma_out
